# revision 9
# baseline (speedup 1.0000x reference)
"""CrossCosineEmbeddingLoss kernel for 8 trn2 NeuronCores.

loss = mean over all (i,j) of: 1 - cos(x_i, y_j) if i==j else relu(cos(x_i, y_j))

Identity:  total = sum_ij relu(sim_ij) + sum_i (1 - sim_ii - relu(sim_ii))
Sharding: rows of x across 8 cores (1024 rows each); y replicated but
rolled per-core so each core's own 1024 rows are y-tiles [0, 8) (the
total loss is invariant to y-row order; this makes the diagonal rows a
compile-time slice under SPMD).

Per-core pipeline (v6):
  - y: SWDGE cast-DMA HBM fp32 -> DRAM bf16 scratch; per-1024-row-group
    xbar transpose-DMA (sprays across all 16 SDMA engines) builds
    yT [d, j] bf16; natural bf16 tiles loaded from the scratch for sumsq
  - sumsq of y rows for 1/||y_j||, applied to per-block row sums at the
    end (relu commutes with positive scaling)
  - x: sumsq -> 1/||x|| -> scale+cast bf16 -> DRAM bounce -> transpose
  - main: 64 j-tiles: 2 bf16 matmuls (yT tile stationary, FWL weight
    loads hide under the stream) -> [128,1024] fp32 PSUM -> fused
    relu+accum split across ACT and DVE
  - final: R * rny, reduce; diagonal correction from natural bf16 tiles
Host combines [128,2] partials from each core.
"""

import numpy as np

import concourse.bacc as bacc
import concourse.bass as bass
import concourse.tile as tile
from concourse import mybir
from concourse.bass_utils import run_bass_kernel_spmd

N, D = 8192, 128
NCORES = 8
SH = N // NCORES          # 1024 rows of x per core
TX = SH // 128            # 8 x-tiles per core
TY = N // 128             # 64 y-tiles
YG = 8                    # y load groups (8 tiles each)

f32 = mybir.dt.float32
bf16 = mybir.dt.bfloat16
AF = mybir.ActivationFunctionType
ALU = mybir.AluOpType
AX = mybir.AxisListType

ACT_TILES = 36              # of 64 main tiles handled by ACT (rest DVE)
CAST_GROUPS = (1, 1, 2, 4)  # y cast-DMA sizes in 1024-row groups


def _reduce_kind(t):
    # spread ACT_TILES evenly over the 64 iterations
    return "act" if (t * ACT_TILES) % TY < ACT_TILES else "dve"


_CACHE = {}


def _build():
    if "nc" in _CACHE:
        return _CACHE["nc"]
    nc = bacc.Bacc("TRN2", target_bir_lowering=False, debug=False,
                   num_devices=NCORES)
    xs_d = nc.dram_tensor("xs", [SH, D], f32, kind="ExternalInput")
    y_d = nc.dram_tensor("y", [N, D], f32, kind="ExternalInput")
    out_d = nc.dram_tensor("out", [128, 2], f32, kind="ExternalOutput")
    ybf_d = nc.dram_tensor("ybf_scr", [N, D], bf16, kind="Internal")
    xh_d = nc.dram_tensor("xh_scr", [SH, D], bf16, kind="Internal")

    with tile.TileContext(nc) as tc:
        with (
            tc.tile_pool(name="singles", bufs=1) as singles,
            tc.tile_pool(name="scrD", bufs=2) as scrD,
        ):
            ybf = singles.tile([128, TY, 128], bf16)    # [j%128, jt, d]
            yT = singles.tile([128, TY, 128], bf16)     # [d, jt, j]
            xnat = singles.tile([128, TX, 128], f32)    # [i%128, it, d]
            xhat = singles.tile([128, TX, 128], bf16)
            xhatT = singles.tile([128, TX, 128], bf16)  # [d, it, i]
            nx2 = singles.tile([128, TX], f32)
            rnx = singles.tile([128, TX], f32)
            ny2 = singles.tile([128, TY], f32)
            rny = singles.tile([128, TY], f32)
            t2y = singles.tile([128, TY], f32)
            t1x = singles.tile([128, TX], f32)
            R = singles.tile([128, TY], f32)
            Ssc = singles.tile([128, TY], f32)
            d2 = singles.tile([128, TX], f32)
            sim_d = singles.tile([128, TX], f32)
            relu_d = singles.tile([128, TX], f32)
            outsb = singles.tile([128, 2], f32)
            warm = singles.tile([128, 8], f32)

            # preload the sqrt activation table set early (overlaps DMA)
            nc.vector.memset(warm[:], 1.0)
            nc.scalar.sqrt(warm[:], warm[:])

            # ---- load x shard: rows r = 128*t + p -> (p, t, d)
            nc.sync.dma_start(
                out=xnat[:], in_=xs_d[:].rearrange("(t p) d -> p t d", p=128))

            # ---- y pipeline: SWDGE casts in geometric chunks (few Q7
            # emissions, early first tiles), per-1024-row transposes
            def y_cast(g0, ng):
                rows = slice(1024 * g0, 1024 * (g0 + ng))
                nc.gpsimd.dma_start(out=ybf_d[rows], in_=y_d[rows])

            def y_post(g):
                rows = slice(1024 * g, 1024 * (g + 1))
                gs = slice(g * TX, (g + 1) * TX)
                nc.sync.dma_start_transpose(
                    out=yT[:, gs, :].rearrange("p t j -> p (t j)"),
                    in_=ybf_d[rows])
                nc.sync.dma_start(
                    out=ybf[:, gs, :],
                    in_=ybf_d[rows].rearrange("(t p) d -> p t d", p=128))

            y_cast(0, 1)
            y_post(0)

            # ---- x norms + scale+cast (DVE) then DRAM-bounce transpose
            for t in range(TX):
                nc.vector.scalar_tensor_tensor(
                    out=scrD.tile([128, 128], f32, tag='sd', name='sd')[:],
                    in0=xnat[:, t, :], scalar=1.0, in1=xnat[:, t, :],
                    op0=ALU.mult, op1=ALU.mult, accum_out=nx2[:, t:t + 1])
            nc.vector.reciprocal(t1x[:], nx2[:])
            nc.scalar.sqrt(rnx[:], t1x[:])   # 1/||x_r||
            for t in range(TX):
                nc.vector.tensor_scalar(
                    out=xhat[:, t, :], in0=xnat[:, t, :],
                    scalar1=rnx[:, t:t + 1], scalar2=None,
                    op0=ALU.mult, op1=ALU.bypass)
            nc.sync.dma_start(
                out=xh_d[:].rearrange("(t p) d -> p t d", p=128), in_=xhat[:])
            nc.sync.dma_start_transpose(
                out=xhatT[:].rearrange("p t i -> p (t i)"), in_=xh_d[:])

            # ---- remaining y chunks + per-tile sumsq
            g = 1
            for ng in CAST_GROUPS[1:]:
                y_cast(g, ng)
                for k in range(ng):
                    y_post(g + k)
                g += ng
            for col in range(TY):
                nc.vector.scalar_tensor_tensor(
                    out=scrD.tile([128, 128], bf16, tag='sq', name='sq')[:],
                    in0=ybf[:, col, :], scalar=1.0, in1=ybf[:, col, :],
                    op0=ALU.mult, op1=ALU.mult,
                    accum_out=ny2[:, col:col + 1])

            # ---- rny = 1/||y_j||
            nc.vector.reciprocal(t2y[:], ny2[:])
            nc.scalar.sqrt(rny[:], t2y[:])

            # ---- main: per j-block bf16 matmuls + fused relu-accumulate
            with tc.tile_pool(name="mpsum", bufs=4, space="PSUM") as mpsum:
                rhs = xhatT[:].rearrange("p a b -> p (a b)")
                for t in range(TY):
                    ps = mpsum.tile([128, 1024], f32, tag="mp")
                    lhsT = yT[:, t, :]
                    nc.tensor.matmul(ps[:, 0:512], lhsT, rhs[:, 0:512])
                    nc.tensor.matmul(ps[:, 512:1024], lhsT, rhs[:, 512:1024])
                    if _reduce_kind(t) == "act":
                        nc.scalar.activation(
                            ps[:], ps[:], AF.Relu, accum_out=R[:, t:t + 1])
                    else:
                        nc.vector.tensor_scalar(
                            out=ps[:], in0=ps[:], scalar1=0.0, scalar2=None,
                            op0=ALU.max, op1=ALU.add,
                            accum_out=R[:, t:t + 1])

            # ---- diagonal: sim_ii for local rows = y tiles [0, TX)
            for t in range(TX):
                nc.vector.scalar_tensor_tensor(
                    out=scrD.tile([128, 128], bf16, tag='dg', name='dg')[:],
                    in0=xhat[:, t, :], scalar=1.0, in1=ybf[:, t, :],
                    op0=ALU.mult, op1=ALU.mult, accum_out=d2[:, t:t + 1])
            nc.vector.tensor_mul(sim_d[:], d2[:], rny[:, 0:TX])
            nc.scalar.activation(relu_d[:], sim_d[:], AF.Relu)
            nc.vector.scalar_tensor_tensor(
                out=scrD.tile([128, TX], f32, tag='df', name='df')[:],
                in0=sim_d[:], scalar=1.0, in1=relu_d[:],
                op0=ALU.mult, op1=ALU.add, accum_out=outsb[:, 1:2])

            # ---- final: scale per-block sums by 1/||y_j|| and total
            nc.vector.tensor_mul(Ssc[:], R[:], rny[:])
            nc.vector.tensor_reduce(out=outsb[:, 0:1], in_=Ssc[:],
                                    axis=AX.X, op=ALU.add)
            nc.sync.dma_start(out=out_d[:], in_=outsb[:])

    nc.compile()
    _CACHE["nc"] = nc
    return nc


def _in_maps(x, y):
    maps = []
    for c in range(NCORES):
        sl = slice(SH * c, SH * (c + 1))
        maps.append({"xs": np.ascontiguousarray(x[sl]),
                     "y": np.roll(y, -SH * c, axis=0)})
    return maps


def _combine(results):
    total = 0.0
    for c in range(NCORES):
        o = results[c]["out"].astype(np.float64)
        total += o[:, 0].sum() - o[:, 1].sum() + SH
    return np.float32(total / (float(N) * float(N)))


def _run(x, y, trace=False):
    nc = _build()
    res = run_bass_kernel_spmd(nc, _in_maps(x, y), list(range(NCORES)),
                               trace=trace)
    return _combine(res.results), res


def kernel(x, y):
    x = np.asarray(x, dtype=np.float32)
    y = np.asarray(y, dtype=np.float32)
    loss, _ = _run(x, y, trace=False)
    return loss


# revision 11
# speedup vs baseline: 1.0181x; 1.0181x over previous
"""CrossCosineEmbeddingLoss kernel for 8 trn2 NeuronCores.

loss = mean over all (i,j) of: 1 - cos(x_i, y_j) if i==j else relu(cos(x_i, y_j))

Identity:  total = sum_ij relu(sim_ij) + sum_i (1 - sim_ii - relu(sim_ii))
Sharding: rows of x across 8 cores (1024 rows each); y replicated, passed
both row-major (for row norms / diagonal) and column-major (for the
matmul operand) — pure layout copies, still fp32.  y rows are rolled
per-core so each core's own 1024 rows are y-tiles [0, 8) (the loss is
invariant to y-row order; this makes the diagonal rows a compile-time
slice under SPMD).

Per-core pipeline (v8):
  - yT fp32 loads in 8 chunks straight into SBUF; GPSIMD (otherwise
    idle) casts fp32 -> bf16 per chunk for the matmul operand
  - y natural fp32 in one DMA; per-tile sumsq on DVE for 1/||y_j||,
    applied to the per-block row sums at the end (relu commutes with
    positive scaling)
  - x: sumsq -> 1/||x|| -> scale+cast bf16 -> DRAM bounce -> xbar
    transpose-DMA
  - main: 64 j-tiles: 2 bf16 matmuls (yT tile stationary, FWL weight
    loads hide under the stream) -> [128,1024] fp32 PSUM -> fused
    relu+accum split across ACT and DVE
  - final: R * rny, reduce; diagonal correction from natural fp32 tiles
Host combines [128,2] partials from each core.
"""

import numpy as np

import concourse.bacc as bacc
import concourse.bass as bass
import concourse.tile as tile
from concourse import mybir
from concourse.bass_utils import run_bass_kernel_spmd

N, D = 8192, 128
NCORES = 8
SH = N // NCORES          # 1024 rows of x per core
TX = SH // 128            # 8 x-tiles per core
TY = N // 128             # 64 y-tiles
YG = 8                    # y chunks (8 tiles each)

f32 = mybir.dt.float32
bf16 = mybir.dt.bfloat16
AF = mybir.ActivationFunctionType
ALU = mybir.AluOpType
AX = mybir.AxisListType

ACT_TILES = 36              # of 64 main tiles handled by ACT (rest DVE)


def _reduce_kind(t):
    # spread ACT_TILES evenly over the 64 iterations
    return "act" if (t * ACT_TILES) % TY < ACT_TILES else "dve"


_CACHE = {}


def _build():
    if "nc" in _CACHE:
        return _CACHE["nc"]
    nc = bacc.Bacc("TRN2", target_bir_lowering=False, debug=False,
                   num_devices=NCORES)
    xs_d = nc.dram_tensor("xs", [SH, D], f32, kind="ExternalInput")
    y_d = nc.dram_tensor("y", [N, D], f32, kind="ExternalInput")
    yt_d = nc.dram_tensor("yt", [D, N], f32, kind="ExternalInput")
    out_d = nc.dram_tensor("out", [128, 2], f32, kind="ExternalOutput")
    xh_d = nc.dram_tensor("xh_scr", [SH, D], bf16, kind="Internal")

    with tile.TileContext(nc) as tc:
        with (
            tc.tile_pool(name="singles", bufs=1) as singles,
            tc.tile_pool(name="scrD", bufs=2) as scrD,
        ):
            yT32 = singles.tile([128, TY, 128], f32)    # [d, jt, j]
            yT = singles.tile([128, TY, 128], bf16)     # [d, jt, j]
            ynat = singles.tile([128, TY, 128], f32)    # [j%128, jt, d]
            xnat = singles.tile([128, TX, 128], f32)    # [i%128, it, d]
            xhat = singles.tile([128, TX, 128], bf16)
            xhatT = singles.tile([128, TX, 128], bf16)  # [d, it, i]
            nx2 = singles.tile([128, TX], f32)
            rnx = singles.tile([128, TX], f32)
            ny2 = singles.tile([128, TY], f32)
            rny = singles.tile([128, TY], f32)
            t2y = singles.tile([128, TY], f32)
            t1x = singles.tile([128, TX], f32)
            R = singles.tile([128, TY], f32)
            Ssc = singles.tile([128, TY], f32)
            d2 = singles.tile([128, TX], f32)
            sim_d = singles.tile([128, TX], f32)
            relu_d = singles.tile([128, TX], f32)
            outsb = singles.tile([128, 2], f32)
            warm = singles.tile([128, 8], f32)

            # preload the sqrt activation table set early (overlaps DMA)
            nc.vector.memset(warm[:], 1.0)
            nc.scalar.sqrt(warm[:], warm[:])

            # ---- load x shard: rows r = 128*t + p -> (p, t, d)
            nc.sync.dma_start(
                out=xnat[:], in_=xs_d[:].rearrange("(t p) d -> p t d", p=128))

            # ---- yT chunks: fp32 load + gpsimd cast to bf16
            for g in range(YG):
                gs = slice(g * TX, (g + 1) * TX)
                nc.sync.dma_start(
                    out=yT32[:, gs, :],
                    in_=yt_d[:, 1024 * g:1024 * (g + 1)]
                    .rearrange("p (t j) -> p t j", j=128))
                nc.gpsimd.tensor_copy(
                    out=yT[:, gs, :].rearrange("p t j -> p (t j)"),
                    in_=yT32[:, gs, :].rearrange("p t j -> p (t j)"))

            # ---- x norms + scale+cast (DVE) then DRAM-bounce transpose
            for t in range(TX):
                nc.vector.scalar_tensor_tensor(
                    out=scrD.tile([128, 128], f32, tag='sd', name='sd')[:],
                    in0=xnat[:, t, :], scalar=1.0, in1=xnat[:, t, :],
                    op0=ALU.mult, op1=ALU.mult, accum_out=nx2[:, t:t + 1])
            nc.vector.reciprocal(t1x[:], nx2[:])
            nc.scalar.sqrt(rnx[:], t1x[:])   # 1/||x_r||
            for t in range(TX):
                nc.vector.tensor_scalar(
                    out=xhat[:, t, :], in0=xnat[:, t, :],
                    scalar1=rnx[:, t:t + 1], scalar2=None,
                    op0=ALU.mult, op1=ALU.bypass)
            nc.sync.dma_start(
                out=xh_d[:].rearrange("(t p) d -> p t d", p=128), in_=xhat[:])
            nc.sync.dma_start_transpose(
                out=xhatT[:].rearrange("p t i -> p (t i)"), in_=xh_d[:])

            # ---- y natural (one DMA) + per-tile sumsq for 1/||y_j||
            nc.sync.dma_start(
                out=ynat[:], in_=y_d[:].rearrange("(t p) d -> p t d", p=128))
            for col in range(TY):
                nc.vector.scalar_tensor_tensor(
                    out=scrD.tile([128, 128], f32, tag='sq', name='sq')[:],
                    in0=ynat[:, col, :], scalar=1.0, in1=ynat[:, col, :],
                    op0=ALU.mult, op1=ALU.mult,
                    accum_out=ny2[:, col:col + 1])
            nc.vector.reciprocal(t2y[:], ny2[:])
            nc.scalar.sqrt(rny[:], t2y[:])

            # ---- main: per j-block bf16 matmuls + fused relu-accumulate
            with tc.tile_pool(name="mpsum", bufs=4, space="PSUM") as mpsum:
                rhs = xhatT[:].rearrange("p a b -> p (a b)")
                for t in range(TY):
                    ps = mpsum.tile([128, 1024], f32, tag="mp")
                    lhsT = yT[:, t, :]
                    nc.tensor.matmul(ps[:, 0:512], lhsT, rhs[:, 0:512])
                    nc.tensor.matmul(ps[:, 512:1024], lhsT, rhs[:, 512:1024])
                    if _reduce_kind(t) == "act":
                        nc.scalar.activation(
                            ps[:], ps[:], AF.Relu, accum_out=R[:, t:t + 1])
                    else:
                        nc.vector.tensor_scalar(
                            out=ps[:], in0=ps[:], scalar1=0.0, scalar2=None,
                            op0=ALU.max, op1=ALU.add,
                            accum_out=R[:, t:t + 1])

            # ---- diagonal: raw dots from fp32 tiles, then scale
            for t in range(TX):
                nc.vector.scalar_tensor_tensor(
                    out=scrD.tile([128, 128], f32, tag='dg', name='dg')[:],
                    in0=xnat[:, t, :], scalar=1.0, in1=ynat[:, t, :],
                    op0=ALU.mult, op1=ALU.mult, accum_out=d2[:, t:t + 1])
            nc.vector.tensor_mul(t1x[:], d2[:], rnx[:])
            nc.vector.tensor_mul(sim_d[:], t1x[:], rny[:, 0:TX])
            nc.scalar.activation(relu_d[:], sim_d[:], AF.Relu)
            nc.vector.scalar_tensor_tensor(
                out=scrD.tile([128, TX], f32, tag='df', name='df')[:],
                in0=sim_d[:], scalar=1.0, in1=relu_d[:],
                op0=ALU.mult, op1=ALU.add, accum_out=outsb[:, 1:2])

            # ---- final: scale per-block sums by 1/||y_j|| and total
            nc.vector.tensor_mul(Ssc[:], R[:], rny[:])
            nc.vector.tensor_reduce(out=outsb[:, 0:1], in_=Ssc[:],
                                    axis=AX.X, op=ALU.add)
            nc.sync.dma_start(out=out_d[:], in_=outsb[:])

    nc.compile()
    _CACHE["nc"] = nc
    return nc


def _in_maps(x, y):
    maps = []
    for c in range(NCORES):
        sl = slice(SH * c, SH * (c + 1))
        yr = np.roll(y, -SH * c, axis=0)
        maps.append({"xs": np.ascontiguousarray(x[sl]),
                     "y": yr,
                     "yt": np.ascontiguousarray(yr.T)})
    return maps


def _combine(results):
    total = 0.0
    for c in range(NCORES):
        o = results[c]["out"].astype(np.float64)
        total += o[:, 0].sum() - o[:, 1].sum() + SH
    return np.float32(total / (float(N) * float(N)))


def _run(x, y, trace=False):
    nc = _build()
    res = run_bass_kernel_spmd(nc, _in_maps(x, y), list(range(NCORES)),
                               trace=trace)
    return _combine(res.results), res


def kernel(x, y):
    x = np.asarray(x, dtype=np.float32)
    y = np.asarray(y, dtype=np.float32)
    loss, _ = _run(x, y, trace=False)
    return loss


# revision 14
# speedup vs baseline: 1.2159x; 1.1943x over previous
"""CrossCosineEmbeddingLoss kernel for 8 trn2 NeuronCores.

loss = mean over all (i,j) of: 1 - cos(x_i, y_j) if i==j else relu(cos(x_i, y_j))

Identity:  total = sum_ij relu(sim_ij) + sum_i (1 - sim_ii - relu(sim_ii))
Sharding: rows of x across 8 cores (1024 rows each); y replicated, passed
both row-major (for row norms / diagonal) and column-major (the matmul
stationary operand) — pure layout copies, still fp32.  y rows are rolled
per-core so each core's own 1024 rows are y-tiles [0, 8) (the loss is
invariant to y-row order; this makes the diagonal rows a compile-time
slice under SPMD).

Per-core pipeline (v9): no dtype casts, no DMA transposes, no SWDGE.
  - yT fp32 chunks load straight into SBUF; matmuls use them as f32r
  - y natural fp32 in one DMA; per-tile sumsq on DVE for 1/||y_j||,
    applied to the per-block row sums at the end (relu commutes with
    positive scaling)
  - x: sumsq -> 1/||x|| -> scale -> 8 PE transposes -> f32r xhatT
  - main: 64 j-tiles: 2 f32r matmuls -> [128,1024] fp32 PSUM -> fused
    relu+accum split across ACT and DVE
  - final: R * rny, reduce; diagonal correction from natural fp32 tiles
Host combines [128,2] partials from each core.
"""

import numpy as np

import concourse.bacc as bacc
import concourse.bass as bass
import concourse.tile as tile
from concourse import mybir
from concourse.bass_utils import run_bass_kernel_spmd
from concourse.masks import make_identity

N, D = 8192, 128
NCORES = 8
SH = N // NCORES          # 1024 rows of x per core
TX = SH // 128            # 8 x-tiles per core
TY = N // 128             # 64 y-tiles
YG = 8                    # y chunks (8 tiles each)

f32 = mybir.dt.float32
f32r = mybir.dt.float32r
bf16 = mybir.dt.bfloat16
AF = mybir.ActivationFunctionType
ALU = mybir.AluOpType
AX = mybir.AxisListType

ACT_TILES = 36              # of 64 main tiles handled by ACT (rest DVE)


def _reduce_kind(t):
    # spread ACT_TILES evenly over the 64 iterations
    return "act" if (t * ACT_TILES) % TY < ACT_TILES else "dve"


_CACHE = {}


def _build():
    if "nc" in _CACHE:
        return _CACHE["nc"]
    nc = bacc.Bacc("TRN2", target_bir_lowering=False, debug=False,
                   num_devices=NCORES)
    xs_d = nc.dram_tensor("xs", [SH, D], f32, kind="ExternalInput")
    y_d = nc.dram_tensor("y", [N, D], f32, kind="ExternalInput")
    yt_d = nc.dram_tensor("yt", [D, N], f32r, kind="ExternalInput")
    out_d = nc.dram_tensor("out", [128, 2], f32, kind="ExternalOutput")

    with tile.TileContext(nc) as tc:
        with (
            tc.tile_pool(name="singles", bufs=1) as singles,
            tc.tile_pool(name="scrD", bufs=2) as scrD,
        ):
            yT32 = singles.tile([128, TY, 128], f32r)   # [d, jt, j]
            ynat = singles.tile([128, TY, 128], f32)    # [j%128, jt, d]
            xnat = singles.tile([128, TX, 128], f32)    # [i%128, it, d]
            xhat = singles.tile([128, TX, 128], f32)
            xhatT = singles.tile([128, TX, 128], f32r)  # [d, it, i]
            ident = singles.tile([128, 128], f32)
            nx2 = singles.tile([128, TX], f32)
            rnx = singles.tile([128, TX], f32)
            ny2 = singles.tile([128, TY], f32)
            rny = singles.tile([128, TY], f32)
            t2y = singles.tile([128, TY], f32)
            t1x = singles.tile([128, TX], f32)
            R = singles.tile([128, TY], f32)
            Ssc = singles.tile([128, TY], f32)
            d2 = singles.tile([128, TX], f32)
            sim_d = singles.tile([128, TX], f32)
            relu_d = singles.tile([128, TX], f32)
            outsb = singles.tile([128, 2], f32)
            warm = singles.tile([128, 8], f32)

            # preload the sqrt activation table set early (overlaps DMA)
            nc.vector.memset(warm[:], 1.0)
            nc.scalar.sqrt(warm[:], warm[:])
            make_identity(nc, ident[:])

            # ---- load x shard: rows r = 128*t + p -> (p, t, d)
            nc.sync.dma_start(
                out=xnat[:], in_=xs_d[:].rearrange("(t p) d -> p t d", p=128))

            # ---- yT chunk 0 + y natural early, rest of yT after
            def yt_chunk(g):
                gs = slice(g * TX, (g + 1) * TX)
                nc.sync.dma_start(
                    out=yT32[:, gs, :],
                    in_=yt_d[:, 1024 * g:1024 * (g + 1)]
                    .rearrange("p (t j) -> p t j", j=128))

            yt_chunk(0)
            nc.sync.dma_start(
                out=ynat[:], in_=y_d[:].rearrange("(t p) d -> p t d", p=128))
            for g in range(1, YG):
                yt_chunk(g)

            # ---- x norms + scale (DVE) then PE transposes
            for t in range(TX):
                nc.vector.scalar_tensor_tensor(
                    out=scrD.tile([128, 128], f32, tag='sd', name='sd')[:],
                    in0=xnat[:, t, :], scalar=1.0, in1=xnat[:, t, :],
                    op0=ALU.mult, op1=ALU.mult, accum_out=nx2[:, t:t + 1])
            nc.vector.reciprocal(t1x[:], nx2[:])
            nc.scalar.sqrt(rnx[:], t1x[:])   # 1/||x_r||
            for t in range(TX):
                nc.vector.tensor_scalar(
                    out=xhat[:, t, :], in0=xnat[:, t, :],
                    scalar1=rnx[:, t:t + 1], scalar2=None,
                    op0=ALU.mult, op1=ALU.bypass)
            with tc.tile_pool(name="tpsum", bufs=1, space="PSUM") as tpsum:
                ptx = tpsum.tile([128, 1024], f32, tag="tp")
                for t in range(TX):
                    nc.tensor.transpose(ptx[:, 128 * t:128 * (t + 1)],
                                        xhat[:, t, :], ident[:])
                nc.vector.tensor_copy(
                    out=xhatT[:].rearrange("p a b -> p (a b)"), in_=ptx[:])

            # ---- y sumsq for 1/||y_j|| (needed only for the final scale)
            for col in range(TY):
                nc.vector.scalar_tensor_tensor(
                    out=scrD.tile([128, 128], f32, tag='sq', name='sq')[:],
                    in0=ynat[:, col, :], scalar=1.0, in1=ynat[:, col, :],
                    op0=ALU.mult, op1=ALU.mult,
                    accum_out=ny2[:, col:col + 1])
            nc.vector.reciprocal(t2y[:], ny2[:])
            nc.scalar.sqrt(rny[:], t2y[:])

            # ---- main: per j-block f32r matmuls + fused relu-accumulate
            with tc.tile_pool(name="mpsum", bufs=3, space="PSUM") as mpsum:
                rhs = xhatT[:].rearrange("p a b -> p (a b)")
                for t in range(TY):
                    ps = mpsum.tile([128, 1024], f32, tag="mp")
                    lhsT = yT32[:, t, :]
                    nc.tensor.matmul(ps[:, 0:512], lhsT, rhs[:, 0:512])
                    nc.tensor.matmul(ps[:, 512:1024], lhsT, rhs[:, 512:1024])
                    if _reduce_kind(t) == "act":
                        nc.scalar.activation(
                            ps[:], ps[:], AF.Relu, accum_out=R[:, t:t + 1])
                    else:
                        nc.vector.tensor_scalar(
                            out=ps[:], in0=ps[:], scalar1=0.0, scalar2=None,
                            op0=ALU.max, op1=ALU.add,
                            accum_out=R[:, t:t + 1])

            # ---- diagonal: raw dots from fp32 tiles, then scale
            for t in range(TX):
                nc.vector.scalar_tensor_tensor(
                    out=scrD.tile([128, 128], f32, tag='dg', name='dg')[:],
                    in0=xnat[:, t, :], scalar=1.0, in1=ynat[:, t, :],
                    op0=ALU.mult, op1=ALU.mult, accum_out=d2[:, t:t + 1])
            nc.vector.tensor_mul(t1x[:], d2[:], rnx[:])
            nc.vector.tensor_mul(sim_d[:], t1x[:], rny[:, 0:TX])
            nc.scalar.activation(relu_d[:], sim_d[:], AF.Relu)
            nc.vector.scalar_tensor_tensor(
                out=scrD.tile([128, TX], f32, tag='df', name='df')[:],
                in0=sim_d[:], scalar=1.0, in1=relu_d[:],
                op0=ALU.mult, op1=ALU.add, accum_out=outsb[:, 1:2])

            # ---- final: scale per-block sums by 1/||y_j|| and total
            nc.vector.tensor_mul(Ssc[:], R[:], rny[:])
            nc.vector.tensor_reduce(out=outsb[:, 0:1], in_=Ssc[:],
                                    axis=AX.X, op=ALU.add)
            nc.sync.dma_start(out=out_d[:], in_=outsb[:])

    nc.compile()
    _CACHE["nc"] = nc
    return nc


def _in_maps(x, y):
    maps = []
    for c in range(NCORES):
        sl = slice(SH * c, SH * (c + 1))
        yr = np.roll(y, -SH * c, axis=0)
        maps.append({"xs": np.ascontiguousarray(x[sl]),
                     "y": yr,
                     "yt": np.ascontiguousarray(yr.T)})
    return maps


def _combine(results):
    total = 0.0
    for c in range(NCORES):
        o = results[c]["out"].astype(np.float64)
        total += o[:, 0].sum() - o[:, 1].sum() + SH
    return np.float32(total / (float(N) * float(N)))


def _run(x, y, trace=False):
    nc = _build()
    res = run_bass_kernel_spmd(nc, _in_maps(x, y), list(range(NCORES)),
                               trace=trace)
    return _combine(res.results), res


def kernel(x, y):
    x = np.asarray(x, dtype=np.float32)
    y = np.asarray(y, dtype=np.float32)
    loss, _ = _run(x, y, trace=False)
    return loss


# revision 15
# speedup vs baseline: 1.3755x; 1.1313x over previous
"""CrossCosineEmbeddingLoss kernel for 8 trn2 NeuronCores.

loss = mean over all (i,j) of: 1 - cos(x_i, y_j) if i==j else relu(cos(x_i, y_j))

Identity:  total = sum_ij relu(sim_ij) + sum_i (1 - sim_ii - relu(sim_ii))
Sharding: rows of x across 8 cores (1024 rows each); y replicated, passed
both row-major (for row norms / diagonal) and column-major (the matmul
stationary operand) — pure layout copies, still fp32.  y rows are rolled
per-core so each core's own 1024 rows are y-tiles [0, 8) (the loss is
invariant to y-row order; this makes the diagonal rows a compile-time
slice under SPMD).

Per-core pipeline (v9): no dtype casts, no DMA transposes, no SWDGE.
  - yT fp32 chunks load straight into SBUF; matmuls use them as f32r
  - y natural fp32 in one DMA; per-tile sumsq on DVE for 1/||y_j||,
    applied to the per-block row sums at the end (relu commutes with
    positive scaling)
  - x: sumsq -> 1/||x|| -> scale -> 8 PE transposes -> f32r xhatT
  - main: 64 j-tiles: 2 f32r matmuls -> [128,1024] fp32 PSUM -> fused
    relu+accum split across ACT and DVE
  - final: R * rny, reduce; diagonal correction from natural fp32 tiles
Host combines [128,2] partials from each core.
"""

import numpy as np

import concourse.bacc as bacc
import concourse.bass as bass
import concourse.tile as tile
from concourse import mybir
from concourse.bass_utils import run_bass_kernel_spmd
from concourse.masks import make_identity

N, D = 8192, 128
NCORES = 8
SH = N // NCORES          # 1024 rows of x per core
TX = SH // 128            # 8 x-tiles per core
TY = N // 128             # 64 y-tiles
YG = 8                    # y chunks (8 tiles each)

f32 = mybir.dt.float32
f32r = mybir.dt.float32r
bf16 = mybir.dt.bfloat16
AF = mybir.ActivationFunctionType
ALU = mybir.AluOpType
AX = mybir.AxisListType

ACT_TILES = 39              # of 64 main tiles handled by ACT (rest DVE)


def _reduce_kind(t):
    # spread ACT_TILES evenly over the 64 iterations
    return "act" if (t * ACT_TILES) % TY < ACT_TILES else "dve"


_CACHE = {}


def _build():
    if "nc" in _CACHE:
        return _CACHE["nc"]
    nc = bacc.Bacc("TRN2", target_bir_lowering=False, debug=False,
                   num_devices=NCORES)
    xs_d = nc.dram_tensor("xs", [SH, D], f32, kind="ExternalInput")
    y_d = nc.dram_tensor("y", [N, D], f32, kind="ExternalInput")
    yt_d = nc.dram_tensor("yt", [D, N], f32r, kind="ExternalInput")
    out_d = nc.dram_tensor("out", [128, 2], f32, kind="ExternalOutput")

    with tile.TileContext(nc) as tc:
        with (
            tc.tile_pool(name="singles", bufs=1) as singles,
            tc.tile_pool(name="scrD", bufs=2) as scrD,
        ):
            yT32 = singles.tile([128, TY, 128], f32r)   # [d, jt, j]
            ynat = singles.tile([128, TY, 128], f32)    # [j%128, jt, d]
            xnat = singles.tile([128, TX, 128], f32)    # [i%128, it, d]
            xhat = singles.tile([128, TX, 128], f32)
            xhatT = singles.tile([128, TX, 128], f32r)  # [d, it, i]
            ident = singles.tile([128, 128], f32)
            nx2 = singles.tile([128, TX], f32)
            rnx = singles.tile([128, TX], f32)
            ny2 = singles.tile([128, TY], f32)
            rny = singles.tile([128, TY], f32)
            t2y = singles.tile([128, TY], f32)
            t1x = singles.tile([128, TX], f32)
            R = singles.tile([128, TY], f32)
            Ssc = singles.tile([128, TY], f32)
            d2 = singles.tile([128, TX], f32)
            sim_d = singles.tile([128, TX], f32)
            relu_d = singles.tile([128, TX], f32)
            outsb = singles.tile([128, 2], f32)
            warm = singles.tile([128, 8], f32)

            # preload the sqrt activation table set early (overlaps DMA)
            nc.vector.memset(warm[:], 1.0)
            nc.scalar.sqrt(warm[:], warm[:])
            make_identity(nc, ident[:])

            # ---- load x shard: rows r = 128*t + p -> (p, t, d)
            nc.sync.dma_start(
                out=xnat[:], in_=xs_d[:].rearrange("(t p) d -> p t d", p=128))

            # ---- yT chunk 0 + y natural early, rest of yT after
            def yt_chunk(g):
                gs = slice(g * TX, (g + 1) * TX)
                nc.sync.dma_start(
                    out=yT32[:, gs, :],
                    in_=yt_d[:, 1024 * g:1024 * (g + 1)]
                    .rearrange("p (t j) -> p t j", j=128))

            yt_chunk(0)
            for g in range(1, YG):
                yt_chunk(g)

            # ---- x norms + scale (DVE) then PE transposes
            for t in range(TX):
                nc.vector.scalar_tensor_tensor(
                    out=scrD.tile([128, 128], f32, tag='sd', name='sd')[:],
                    in0=xnat[:, t, :], scalar=1.0, in1=xnat[:, t, :],
                    op0=ALU.mult, op1=ALU.mult, accum_out=nx2[:, t:t + 1])
            nc.vector.reciprocal(t1x[:], nx2[:])
            nc.scalar.sqrt(rnx[:], t1x[:])   # 1/||x_r||
            for t in range(TX):
                nc.vector.tensor_scalar(
                    out=xhat[:, t, :], in0=xnat[:, t, :],
                    scalar1=rnx[:, t:t + 1], scalar2=None,
                    op0=ALU.mult, op1=ALU.bypass)
            with tc.tile_pool(name="tpsum", bufs=1, space="PSUM") as tpsum:
                ptx = tpsum.tile([128, 1024], f32, tag="tp")
                for t in range(TX):
                    nc.tensor.transpose(ptx[:, 128 * t:128 * (t + 1)],
                                        xhat[:, t, :], ident[:])
                nc.vector.tensor_copy(
                    out=xhatT[:].rearrange("p a b -> p (a b)"), in_=ptx[:])

            # ---- main: per j-block f32r matmuls + fused relu-accumulate
            with tc.tile_pool(name="mpsum", bufs=3, space="PSUM") as mpsum:
                rhs = xhatT[:].rearrange("p a b -> p (a b)")
                for t in range(TY):
                    ps = mpsum.tile([128, 1024], f32, tag="mp")
                    lhsT = yT32[:, t, :]
                    nc.tensor.matmul(ps[:, 0:512], lhsT, rhs[:, 0:512])
                    nc.tensor.matmul(ps[:, 512:1024], lhsT, rhs[:, 512:1024])
                    if _reduce_kind(t) == "act":
                        nc.scalar.activation(
                            ps[:], ps[:], AF.Relu, accum_out=R[:, t:t + 1])
                    else:
                        nc.vector.tensor_scalar(
                            out=ps[:], in0=ps[:], scalar1=0.0, scalar2=None,
                            op0=ALU.max, op1=ALU.add,
                            accum_out=R[:, t:t + 1])

            # ---- y natural + sumsq for 1/||y_j|| (only needed at the end)
            nc.sync.dma_start(
                out=ynat[:], in_=y_d[:].rearrange("(t p) d -> p t d", p=128))
            for col in range(TY):
                nc.vector.scalar_tensor_tensor(
                    out=scrD.tile([128, 128], f32, tag='sq', name='sq')[:],
                    in0=ynat[:, col, :], scalar=1.0, in1=ynat[:, col, :],
                    op0=ALU.mult, op1=ALU.mult,
                    accum_out=ny2[:, col:col + 1])
            nc.vector.reciprocal(t2y[:], ny2[:])
            nc.scalar.sqrt(rny[:], t2y[:])

            # ---- diagonal: raw dots from fp32 tiles, then scale
            for t in range(TX):
                nc.vector.scalar_tensor_tensor(
                    out=scrD.tile([128, 128], f32, tag='dg', name='dg')[:],
                    in0=xnat[:, t, :], scalar=1.0, in1=ynat[:, t, :],
                    op0=ALU.mult, op1=ALU.mult, accum_out=d2[:, t:t + 1])
            nc.vector.tensor_mul(t1x[:], d2[:], rnx[:])
            nc.vector.tensor_mul(sim_d[:], t1x[:], rny[:, 0:TX])
            nc.scalar.activation(relu_d[:], sim_d[:], AF.Relu)
            nc.vector.scalar_tensor_tensor(
                out=scrD.tile([128, TX], f32, tag='df', name='df')[:],
                in0=sim_d[:], scalar=1.0, in1=relu_d[:],
                op0=ALU.mult, op1=ALU.add, accum_out=outsb[:, 1:2])

            # ---- final: scale per-block sums by 1/||y_j|| and total
            nc.vector.tensor_mul(Ssc[:], R[:], rny[:])
            nc.vector.tensor_reduce(out=outsb[:, 0:1], in_=Ssc[:],
                                    axis=AX.X, op=ALU.add)
            nc.sync.dma_start(out=out_d[:], in_=outsb[:])

    nc.compile()
    _CACHE["nc"] = nc
    return nc


def _in_maps(x, y):
    maps = []
    for c in range(NCORES):
        sl = slice(SH * c, SH * (c + 1))
        yr = np.roll(y, -SH * c, axis=0)
        maps.append({"xs": np.ascontiguousarray(x[sl]),
                     "y": yr,
                     "yt": np.ascontiguousarray(yr.T)})
    return maps


def _combine(results):
    total = 0.0
    for c in range(NCORES):
        o = results[c]["out"].astype(np.float64)
        total += o[:, 0].sum() - o[:, 1].sum() + SH
    return np.float32(total / (float(N) * float(N)))


def _run(x, y, trace=False):
    nc = _build()
    res = run_bass_kernel_spmd(nc, _in_maps(x, y), list(range(NCORES)),
                               trace=trace)
    return _combine(res.results), res


def kernel(x, y):
    x = np.asarray(x, dtype=np.float32)
    y = np.asarray(y, dtype=np.float32)
    loss, _ = _run(x, y, trace=False)
    return loss


# revision 18
# speedup vs baseline: 1.4311x; 1.0404x over previous
"""CrossCosineEmbeddingLoss kernel for 8 trn2 NeuronCores.

loss = mean over all (i,j) of: 1 - cos(x_i, y_j) if i==j else relu(cos(x_i, y_j))

Identity:  total = sum_ij relu(sim_ij) + sum_i (1 - sim_ii - relu(sim_ii))
Sharding: rows of x across 8 cores (1024 rows each); y replicated, passed
row-major sliced (yd, for row norms / diagonal) and column-major (yt, the
matmul stationary operand) — pure layout copies, still fp32.

Per-core pipeline (v11): no dtype casts, no DMA transposes, no SWDGE.
  - yT fp32 chunks load straight into SBUF; matmuls use them as f32r
  - 1/||y_j|| computed distributed: each core does sumsq+rsqrt of its own
    1024 rows (from yd) and a 4KB AllGather shares all 8 slices; applied
    to the per-block row sums at the end (relu commutes with positive
    scaling), so the collective is fully latency-tolerant
  - x: sumsq -> 1/||x|| -> scale -> 8 PE transposes -> f32r xhatT
  - main: 64 j-tiles: 2 f32r matmuls -> [128,1024] fp32 PSUM -> fused
    relu+accum split across ACT and DVE
  - final: R * rny, reduce; diagonal correction from yd fp32 tiles
Host combines [128,2] partials from each core.
"""

import numpy as np

import concourse.bacc as bacc
import concourse.bass as bass
import concourse.tile as tile
from concourse import mybir
from concourse.bass_utils import run_bass_kernel_spmd
from concourse.masks import make_identity

N, D = 8192, 128
NCORES = 8
SH = N // NCORES          # 1024 rows of x per core
TX = SH // 128            # 8 x-tiles per core
TY = N // 128             # 64 y-tiles
YG = 8                    # y chunks (8 tiles each)

f32 = mybir.dt.float32
f32r = mybir.dt.float32r
AF = mybir.ActivationFunctionType
ALU = mybir.AluOpType
AX = mybir.AxisListType

ACT_TILES = 34              # of 64 main tiles handled by ACT (rest DVE)


def _reduce_kind(t):
    # spread ACT_TILES evenly over the 64 iterations
    return "act" if (t * ACT_TILES) % TY < ACT_TILES else "dve"


_CACHE = {}


def _build():
    if "nc" in _CACHE:
        return _CACHE["nc"]
    nc = bacc.Bacc("TRN2", target_bir_lowering=False, debug=False,
                   num_devices=NCORES)
    xs_d = nc.dram_tensor("xs", [SH, D], f32, kind="ExternalInput")
    yd_d = nc.dram_tensor("yd", [SH, D], f32, kind="ExternalInput")
    yt_d = nc.dram_tensor("yt", [D, N], f32r, kind="ExternalInput")
    out_d = nc.dram_tensor("out", [128, 2], f32, kind="ExternalOutput")
    rl_d = nc.dram_tensor("rny_loc", [TX, 128], f32, kind="Internal")
    ra_d = nc.dram_tensor("rny_all", [TY, 128], f32, kind="Internal")

    with tile.TileContext(nc) as tc:
        with (
            tc.tile_pool(name="singles", bufs=1) as singles,
            tc.tile_pool(name="scrD", bufs=2) as scrD,
        ):
            yT32 = singles.tile([128, TY, 128], f32r)   # [d, jt, j]
            ydnat = singles.tile([128, TX, 128], f32)   # local y rows
            xnat = singles.tile([128, TX, 128], f32)    # [i%128, it, d]
            xhat = singles.tile([128, TX, 128], f32)
            xhatT = singles.tile([128, TX, 128], f32r)  # [d, it, i]
            ident = singles.tile([128, 128], f32)
            nx2 = singles.tile([128, TX], f32)
            rnx = singles.tile([128, TX], f32)
            nyd2 = singles.tile([128, TX], f32)
            rnyd = singles.tile([128, TX], f32)
            rnydT = singles.tile([128, 128], f32)   # rows 0:TX used
            rnyaT = singles.tile([64, 128], f32)    # gathered, t-major
            rny = singles.tile([128, TY], f32)
            t1x = singles.tile([128, TX], f32)
            R = singles.tile([128, TY], f32)
            Ssc = singles.tile([128, TY], f32)
            d2 = singles.tile([128, TX], f32)
            sim_d = singles.tile([128, TX], f32)
            relu_d = singles.tile([128, TX], f32)
            outsb = singles.tile([128, 2], f32)
            warm = singles.tile([128, 8], f32)

            # preload the sqrt activation table set early (overlaps DMA)
            nc.vector.memset(warm[:], 1.0)
            nc.scalar.sqrt(warm[:], warm[:])
            make_identity(nc, ident[:])

            # ---- small loads first: x shard + local y rows
            nc.sync.dma_start(
                out=xnat[:], in_=xs_d[:].rearrange("(t p) d -> p t d", p=128))
            nc.sync.dma_start(
                out=ydnat[:], in_=yd_d[:].rearrange("(t p) d -> p t d", p=128))

            # ---- yT chunks
            for g in range(YG):
                gs = slice(g * TX, (g + 1) * TX)
                nc.sync.dma_start(
                    out=yT32[:, gs, :],
                    in_=yt_d[:, 1024 * g:1024 * (g + 1)]
                    .rearrange("p (t j) -> p t j", j=128))

            # ---- x norms + scale (DVE) then PE transposes
            for t in range(TX):
                nc.vector.scalar_tensor_tensor(
                    out=scrD.tile([128, 128], f32, tag='sd', name='sd')[:],
                    in0=xnat[:, t, :], scalar=1.0, in1=xnat[:, t, :],
                    op0=ALU.mult, op1=ALU.mult, accum_out=nx2[:, t:t + 1])
            nc.vector.reciprocal(t1x[:], nx2[:])
            nc.scalar.sqrt(rnx[:], t1x[:])   # 1/||x_r||
            for t in range(TX):
                nc.vector.tensor_scalar(
                    out=xhat[:, t, :], in0=xnat[:, t, :],
                    scalar1=rnx[:, t:t + 1], scalar2=None,
                    op0=ALU.mult, op1=ALU.bypass)
            with tc.tile_pool(name="tpsum", bufs=1, space="PSUM") as tpsum:
                ptx = tpsum.tile([128, 1024], f32, tag="tp")
                for t in range(TX):
                    nc.tensor.transpose(ptx[:, 128 * t:128 * (t + 1)],
                                        xhat[:, t, :], ident[:])
                nc.vector.tensor_copy(
                    out=xhatT[:].rearrange("p a b -> p (a b)"), in_=ptx[:])

                # ---- local y norms -> rnyd [128, TX]; transpose + gather
                for t in range(TX):
                    nc.vector.scalar_tensor_tensor(
                        out=scrD.tile([128, 128], f32, tag='sq', name='sq')[:],
                        in0=ydnat[:, t, :], scalar=1.0, in1=ydnat[:, t, :],
                        op0=ALU.mult, op1=ALU.mult,
                        accum_out=nyd2[:, t:t + 1])
                nc.vector.reciprocal(t1x[:], nyd2[:])
                nc.scalar.sqrt(rnyd[:], t1x[:])  # 1/||y_i|| local rows
                ptr = tpsum.tile([128, 128], f32, tag="tq")
                nc.tensor.transpose(ptr[0:TX, :], rnyd[:], ident[:])
                nc.vector.tensor_copy(out=rnydT[0:TX, :], in_=ptr[0:TX, :])
            nc.sync.dma_start(out=rl_d[:], in_=rnydT[0:TX, :])
            nc.gpsimd.collective_compute(
                kind="AllGather", op=ALU.bypass,
                replica_groups=[list(range(NCORES))],
                ins=[rl_d[:]], outs=[ra_d[:]])

            # ---- main: per j-block f32r matmuls + fused relu-accumulate
            with tc.tile_pool(name="mpsum", bufs=3, space="PSUM") as mpsum:
                rhs = xhatT[:].rearrange("p a b -> p (a b)")
                for t in range(TY):
                    ps = mpsum.tile([128, 1024], f32, tag="mp")
                    lhsT = yT32[:, t, :]
                    nc.tensor.matmul(ps[:, 0:512], lhsT, rhs[:, 0:512])
                    nc.tensor.matmul(ps[:, 512:1024], lhsT, rhs[:, 512:1024])
                    if _reduce_kind(t) == "act":
                        nc.scalar.activation(
                            ps[:], ps[:], AF.Relu, accum_out=R[:, t:t + 1])
                    else:
                        nc.vector.tensor_scalar(
                            out=ps[:], in0=ps[:], scalar1=0.0, scalar2=None,
                            op0=ALU.max, op1=ALU.add,
                            accum_out=R[:, t:t + 1])

            # ---- gathered rny: load t-major, PE transpose to [128, TY]
            nc.sync.dma_start(out=rnyaT[:], in_=ra_d[:])
            with tc.tile_pool(name="gpsum", bufs=1, space="PSUM") as gpsum:
                ptg = gpsum.tile([128, 64], f32, tag="tg")
                nc.tensor.transpose(ptg[:], rnyaT[:], ident[0:64, 0:64])
                nc.vector.tensor_copy(out=rny[:], in_=ptg[:])

            # ---- diagonal: raw dots from fp32 tiles, then scale
            for t in range(TX):
                nc.vector.scalar_tensor_tensor(
                    out=scrD.tile([128, 128], f32, tag='dg', name='dg')[:],
                    in0=xnat[:, t, :], scalar=1.0, in1=ydnat[:, t, :],
                    op0=ALU.mult, op1=ALU.mult, accum_out=d2[:, t:t + 1])
            nc.vector.tensor_mul(t1x[:], d2[:], rnx[:])
            nc.vector.tensor_mul(sim_d[:], t1x[:], rnyd[:])
            nc.scalar.activation(relu_d[:], sim_d[:], AF.Relu)
            nc.vector.scalar_tensor_tensor(
                out=scrD.tile([128, TX], f32, tag='df', name='df')[:],
                in0=sim_d[:], scalar=1.0, in1=relu_d[:],
                op0=ALU.mult, op1=ALU.add, accum_out=outsb[:, 1:2])

            # ---- final: scale per-block sums by 1/||y_j|| and total
            nc.vector.tensor_mul(Ssc[:], R[:], rny[:])
            nc.vector.tensor_reduce(out=outsb[:, 0:1], in_=Ssc[:],
                                    axis=AX.X, op=ALU.add)
            nc.sync.dma_start(out=out_d[:], in_=outsb[:])

    nc.compile()
    _CACHE["nc"] = nc
    return nc


def _in_maps(x, y):
    yt = np.ascontiguousarray(y.T)
    maps = []
    for c in range(NCORES):
        sl = slice(SH * c, SH * (c + 1))
        maps.append({"xs": np.ascontiguousarray(x[sl]),
                     "yd": np.ascontiguousarray(y[sl]),
                     "yt": yt})
    return maps


def _combine(results):
    total = 0.0
    for c in range(NCORES):
        o = results[c]["out"].astype(np.float64)
        total += o[:, 0].sum() - o[:, 1].sum() + SH
    return np.float32(total / (float(N) * float(N)))


def _run(x, y, trace=False):
    nc = _build()
    res = run_bass_kernel_spmd(nc, _in_maps(x, y), list(range(NCORES)),
                               trace=trace)
    return _combine(res.results), res


def kernel(x, y):
    x = np.asarray(x, dtype=np.float32)
    y = np.asarray(y, dtype=np.float32)
    loss, _ = _run(x, y, trace=False)
    return loss
